# revision 19
# baseline (speedup 1.0000x reference)
"""Trainium2 Bass kernel for the CoarseGraining problem.

Computes y[i, b] = heg[b] * sum_j wrho[j] * exp(-beta[j, b] * d2[i, j])
with d2 the pairwise squared distances between out_coords (i) and coords (j).

Strategy (8 NeuronCores, SPMD):
  - Shard the j (source) axis: each core owns 1024 sources and reduces them
    over ALL 8192 output points; host sums the 8 partial results at the end.
  - Per-core layout: partitions = j (chunks of 128), free axis = i.
  - The tiny MLP (beta), wrho, heg and the squared norms are precomputed on
    the host in float64; beta/wrho enter the device kernel only through the
    per-partition scale/bias operands of the activation instruction.
  - Device pipeline per chunk c (128 sources):
      1. PE:  K=4 fp32 matmul  P'[j, i] = c_j . x_i - ri2[i]/2    (16 x N=512)
      2. DVE: clamp  d2s = min(P', rj2[j]/2)  (== enforcing d2 >= 0)
      3. ACT (per basis b): E = exp(2*beta[j,b] * d2s + bias[j,b]) -> fp16
         where bias = -beta*rj2 + ln(1024*wrho); ONE (128, 8192) op.
      4. PE:  reduce over j: lhsT = E[:, 128-block] (stationary), rhs = ones
         column -> psum column y[(b, blk)], accumulated over chunks in PSUM.
"""

import numpy as np
from contextlib import ExitStack

N_CORES = 8
N_SRC = 8192
M_OUT = 8192
NB = 16
EPS = 1e-4
LOG2 = 0.6931471805599453
SCALE = 1024.0

_CACHE = {}
_LAST_RUN = {}


def _build_nc(n_src_pc, m_out, nb):
    import concourse.bass as bass
    import concourse.tile as tile
    from concourse import bacc, mybir

    f32 = mybir.dt.float32
    f16 = mybir.dt.float16

    C = n_src_pc // 128          # j-chunks per core
    NIC = m_out // 512           # d2 matmul slices
    NBLK = m_out // 128          # reduce blocks (i blocks of 128)

    nc = bacc.Bacc("TRN2", target_bir_lowering=False, debug=False)
    # geom cols 0:m_out   -> rhs rows [x; y; z; 1; -ri2/2]
    # geom cols m_out:    -> lhsT rows [cx; cy; cz; -rj2/2; 1]
    geom_d = nc.dram_tensor("geom", [5, m_out + n_src_pc], f32, kind="ExternalInput")
    # coef: [2*beta (C*nb) | ln(SCALE*wrho) (C)] per 128-partition layout
    coef_d = nc.dram_tensor("coef", [128, C * nb + C], f32, kind="ExternalInput")
    y_d = nc.dram_tensor("yout", [128, nb * NBLK], f32, kind="ExternalOutput")

    with ExitStack() as ctx:
        tc = ctx.enter_context(tile.TileContext(nc))
        consts = ctx.enter_context(tc.tile_pool(name="consts", bufs=1))
        d2pool = ctx.enter_context(tc.tile_pool(name="d2p", bufs=2))
        epool = ctx.enter_context(tc.tile_pool(name="ep", bufs=3))
        ppool = ctx.enter_context(tc.tile_pool(name="pp", bufs=4, space="PSUM"))
        ypool = ctx.enter_context(tc.tile_pool(name="yp", bufs=1, space="PSUM"))
        opool = ctx.enter_context(tc.tile_pool(name="op", bufs=1))

        geom_sb = consts.tile([5, m_out + n_src_pc], f32)
        nc.sync.dma_start(out=geom_sb[:], in_=geom_d.ap())
        rhs_sb = geom_sb[:, 0:m_out]
        lhs_sb = geom_sb[:, m_out:m_out + n_src_pc]
        coef_sb = consts.tile([128, C * nb + C], f32)
        nc.sync.dma_start(out=coef_sb[:], in_=coef_d.ap())
        b2_sb = coef_sb[:, 0:C * nb]
        bi_sb = coef_sb[:, C * nb:C * nb + C]
        # scratch for wait-absorbing dummy ACT ops (AP-operand instructions
        # only have a single sync-wait slot in the hardware encoding)
        ascr = consts.tile([128, 1], f32)
        nc.scalar.copy(out=ascr[:], in_=coef_sb[:, 0:1])

        # ones tile: column 0 is the reduce-matmul rhs; the whole tile is the
        # dummy zeroing matmul's lhsT (so later matmuls never re-wait on it).
        ol_sb = consts.tile([128, 128], f16)
        nc.vector.memset(ol_sb[:], 1.0)
        zrhs_sb = consts.tile([128, min(512, nb * NBLK)], f16)
        nc.vector.memset(zrhs_sb[:], 0.0)

        y_ps = ypool.tile([128, nb * NBLK], f32)

        # Zero-initialize y_ps with whole-bank dummy matmuls (start=True
        # clears has_written for the entire bank); all real reduce matmuls
        # then accumulate with start=False, making their order irrelevant.
        n_ycols = nb * NBLK
        for col0 in range(0, n_ycols, 512):
            w = min(512, n_ycols - col0)
            nc.tensor.matmul(
                out=y_ps[:, col0:col0 + w],
                lhsT=ol_sb[:],
                rhs=zrhs_sb[:, :w],
                start=True,
                stop=False,
            )

        d2_tiles = {}

        def emit_d2(c, ic):
            pt = ppool.tile([128, 512], f32, tag="d2psum")
            nc.tensor.matmul(
                out=pt[:],
                lhsT=lhs_sb[:, c * 128:(c + 1) * 128],
                rhs=rhs_sb[:, ic * 512:(ic + 1) * 512],
                start=True,
                stop=True,
            )
            # pt = -d2/2; clamp d2 >= 0  <=>  pt <= 0 (immediate scalar)
            nc.vector.tensor_scalar_min(
                d2_tiles[c][:, ic * 512:(ic + 1) * 512], pt[:], 0.0
            )

        d2_tiles[0] = d2pool.tile([128, m_out], f32, tag="d2s", name="d2s0")
        for ic in range(NIC):
            emit_d2(0, ic)

        for c in range(C):
            if c + 1 < C:
                d2_tiles[c + 1] = d2pool.tile(
                    [128, m_out], f32, tag="d2s", name=f"d2s{c + 1}"
                )
            # absorber: advance ACT's observed DVE tick past this chunk's
            # clamps, so the real activations below carry at most one wait
            nc.scalar.copy(out=ascr[:], in_=d2_tiles[c][:, m_out - 1:m_out])
            for b in range(nb):
                col = c * nb + b
                e = epool.tile([128, m_out], f16, tag="e")
                nc.scalar.activation(
                    out=e[:],
                    in_=d2_tiles[c][:],
                    func=mybir.ActivationFunctionType.Exp,
                    bias=bi_sb[:, c:c + 1],
                    scale=b2_sb[:, col:col + 1],
                )
                for blk in range(NBLK):
                    nc.tensor.matmul(
                        out=y_ps[:, b * NBLK + blk: b * NBLK + blk + 1],
                        lhsT=e[:, blk * 128:(blk + 1) * 128],
                        rhs=ol_sb[:, 0:1],
                        start=False,
                        stop=False,
                    )
                # interleave next chunk's d2 so PE/ACT never stall on it
                if c + 1 < C and b < NIC:
                    emit_d2(c + 1, b)
            del d2_tiles[c]

        # Close the accumulation groups: whole-bank +0 matmuls with stop=True.
        # The full-bank WAW overlap orders these after every real reduce matmul.
        for col0 in range(0, n_ycols, 512):
            w = min(512, n_ycols - col0)
            nc.tensor.matmul(
                out=y_ps[:, col0:col0 + w],
                lhsT=ol_sb[:],
                rhs=zrhs_sb[:, :w],
                start=False,
                stop=True,
            )

        y_sb = opool.tile([128, nb * NBLK], f32)
        nc.vector.tensor_copy(out=y_sb[:], in_=y_ps[:])
        nc.sync.dma_start(out=y_d.ap(), in_=y_sb[:])

    nc.compile()
    return nc


def _host_precompute(rho, gamma, coords, weights, out_coords, w1, b1, w2, b2):
    """Float64 host-side precompute of the tiny MLP and derived vectors."""
    rho = rho.astype(np.float64)
    gamma = gamma.astype(np.float64)
    coords64 = coords.astype(np.float64)
    weights64 = weights.astype(np.float64)
    oc64 = out_coords.astype(np.float64)
    w1, b1, w2, b2 = (a.astype(np.float64) for a in (w1, b1, w2, b2))

    def log_cosh(z):
        a = np.abs(z)
        return a + np.log1p(np.exp(-2.0 * a)) - LOG2

    def field_embed(x):
        return np.tanh(x @ w1 + b1) @ w2 + b2

    s2 = gamma / (4.0 * (3.0 * np.pi ** 2) ** (2.0 / 3.0) * rho ** (8.0 / 3.0))
    x = np.log(s2 + EPS)[:, None]
    exponent = log_cosh(field_embed(x))                      # (N, NB)
    heg = log_cosh(field_embed(np.zeros((1, 1)))) ** 1.5     # (1, NB)
    beta = np.pi * (rho[:, None] / 2.0) ** (2.0 / 3.0) * exponent  # (N, NB)
    wrho = weights64 * rho                                   # (N,)
    rj2 = (coords64 ** 2).sum(axis=1)                        # (N,)
    ri2 = (oc64 ** 2).sum(axis=1)                            # (M,)
    lnw = np.log(SCALE * np.maximum(wrho, 1e-300))           # (N,)
    return beta, wrho, heg[0], rj2, ri2, lnw, coords64, oc64


def kernel(rho, gamma, coords, weights, out_coords, w1, b1, w2, b2):
    from concourse.bass_utils import run_bass_kernel_spmd

    n_src = coords.shape[0]
    m_out = out_coords.shape[0]
    nb = w2.shape[1]
    n_src_pc = n_src // N_CORES
    C = n_src_pc // 128
    NBLK = m_out // 128

    beta, wrho, heg, rj2, ri2, lnw, coords64, oc64 = _host_precompute(
        rho, gamma, coords, weights, out_coords, w1, b1, w2, b2
    )

    key = (n_src_pc, m_out, nb)
    if key not in _CACHE:
        _CACHE[key] = _build_nc(n_src_pc, m_out, nb)
    nc = _CACHE[key]

    rhs_aug = np.concatenate(
        [oc64.T, np.ones((1, m_out)), (-0.5 * ri2)[None, :]], axis=0
    )                                                        # (5, M)

    in_maps = []
    for k in range(N_CORES):
        js = slice(k * n_src_pc, (k + 1) * n_src_pc)
        lhs_aug = np.concatenate(
            [coords64[js].T, (-0.5 * rj2[js])[None, :], np.ones((1, n_src_pc))],
            axis=0,
        )                                                    # (5, n_pc)
        geom = np.concatenate([rhs_aug, lhs_aug], axis=1).astype(np.float32)
        # (128, C*nb) with column c*nb+b -> source j = k*n_pc + c*128 + p
        beta2 = (2.0 * beta[js]).reshape(C, 128, nb).transpose(1, 0, 2).reshape(
            128, C * nb
        )
        lnwv = lnw[js].reshape(C, 128).T                     # (128, C)
        coef = np.concatenate([beta2, lnwv], axis=1).astype(np.float32)
        in_maps.append(
            {
                "geom": np.ascontiguousarray(geom),
                "coef": np.ascontiguousarray(coef),
            }
        )

    res = run_bass_kernel_spmd(nc, in_maps, core_ids=list(range(N_CORES)))
    _LAST_RUN["nc"] = nc
    _LAST_RUN["in_maps"] = in_maps
    _LAST_RUN["results"] = res

    ytot = np.zeros((m_out, nb), dtype=np.float64)
    for k in range(N_CORES):
        arr = res.results[k]["yout"]                         # (128, nb*NBLK)
        part = arr.reshape(128, nb, NBLK).transpose(2, 0, 1).reshape(m_out, nb)
        ytot += part.astype(np.float64)
    y = ytot * heg[None, :] / SCALE
    return y.astype(np.float32)


# revision 27
# speedup vs baseline: 1.0025x; 1.0025x over previous
"""Trainium2 Bass kernel for the CoarseGraining problem.

Computes y[i, b] = heg[b] * sum_j wrho[j] * exp(-beta[j, b] * d2[i, j])
with d2 the pairwise squared distances between out_coords (i) and coords (j).

Strategy (8 NeuronCores, SPMD):
  - Shard the j (source) axis: each core owns 1024 sources and reduces them
    over ALL 8192 output points; host sums the 8 partial results at the end.
  - Per-core layout: partitions = j (chunks of 128), free axis = i.
  - The tiny MLP (beta), wrho, heg and the squared norms are precomputed on
    the host in float64; beta/wrho enter the device kernel only through the
    per-partition scale/bias operands of the activation instruction.
  - Device pipeline per chunk c (128 sources):
      1. PE:  K=4 fp32 matmul  P'[j, i] = c_j . x_i - ri2[i]/2    (16 x N=512)
      2. DVE: clamp  d2s = min(P', rj2[j]/2)  (== enforcing d2 >= 0)
      3. ACT (per basis b): E = exp(2*beta[j,b] * d2s + bias[j,b]) -> fp16
         where bias = -beta*rj2 + ln(1024*wrho); ONE (128, 8192) op.
      4. PE:  reduce over j: lhsT = E[:, 128-block] (stationary), rhs = ones
         column -> psum column y[(b, blk)], accumulated over chunks in PSUM.
"""

import numpy as np
from contextlib import ExitStack

N_CORES = 8
N_SRC = 8192
M_OUT = 8192
NB = 16
EPS = 1e-4
LOG2 = 0.6931471805599453
SCALE = 1024.0

_CACHE = {}
_LAST_RUN = {}


def _build_nc(n_src_pc, m_out, nb):
    import concourse.bass as bass
    import concourse.tile as tile
    from concourse import bacc, mybir

    f32 = mybir.dt.float32
    f32r = mybir.dt.float32r
    f16 = mybir.dt.float16

    C = n_src_pc // 128          # j-chunks per core
    NIC = m_out // 512           # d2 matmul slices
    NBLK = m_out // 128          # reduce blocks (i blocks of 128)

    nc = bacc.Bacc("TRN2", target_bir_lowering=False, debug=False)
    # geom cols 0:m_out   -> rhs rows [x; y; z; 1; -ri2/2]
    # geom cols m_out:    -> lhsT rows [cx; cy; cz; -rj2/2; 1]
    geom_d = nc.dram_tensor("geom", [5, m_out + n_src_pc], f32, kind="ExternalInput")
    # coef: [2*beta (C*nb) | ln(SCALE*wrho) (C)] per 128-partition layout
    coef_d = nc.dram_tensor("coef", [128, C * nb + C], f32, kind="ExternalInput")
    y_d = nc.dram_tensor("yout", [128, nb * NBLK], f32, kind="ExternalOutput")

    with ExitStack() as ctx:
        tc = ctx.enter_context(tile.TileContext(nc))
        consts = ctx.enter_context(tc.tile_pool(name="consts", bufs=1))
        d2pool = ctx.enter_context(tc.tile_pool(name="d2p", bufs=2))
        epool = ctx.enter_context(tc.tile_pool(name="ep", bufs=3))
        ppool = ctx.enter_context(tc.tile_pool(name="pp", bufs=4, space="PSUM"))
        ypool = ctx.enter_context(tc.tile_pool(name="yp", bufs=1, space="PSUM"))
        opool = ctx.enter_context(tc.tile_pool(name="op", bufs=1))

        geom_sb = consts.tile([5, m_out + n_src_pc], f32)
        nc.sync.dma_start(out=geom_sb[:], in_=geom_d.ap())
        rhs_sb = geom_sb[:, 0:m_out]
        lhs_sb = geom_sb[:, m_out:m_out + n_src_pc]
        coef_sb = consts.tile([128, C * nb + C], f32)
        nc.sync.dma_start(out=coef_sb[:], in_=coef_d.ap())
        b2_sb = coef_sb[:, 0:C * nb]
        bi_sb = coef_sb[:, C * nb:C * nb + C]
        # ones tile memset happens below; emit an ACT op that depends on
        # nothing slow so the exp table load fires at t~0, then a scratch
        # ACT op per dependency source to absorb sync waits (AP-operand
        # instructions only have a single sync-wait slot in the encoding)
        ascr = consts.tile([128, 1], f32)

        # ones tile: column 0 is the reduce-matmul rhs; the whole tile is the
        # dummy zeroing matmul's lhsT (so later matmuls never re-wait on it).
        ol_sb = consts.tile([128, 128], f16)
        nc.vector.memset(ol_sb[:], 1.0)
        zrhs_sb = consts.tile([128, min(512, nb * NBLK)], f16)
        nc.vector.memset(zrhs_sb[:], 0.0)
        nc.scalar.copy(out=ascr[:], in_=ol_sb[:, 0:1])   # early table load
        nc.scalar.copy(out=ascr[:], in_=coef_sb[:, 0:1])  # absorb coef DMA wait

        y_ps = ypool.tile([128, nb * NBLK], f32)

        # Zero-initialize y_ps with whole-bank dummy matmuls (start=True
        # clears has_written for the entire bank); all real reduce matmuls
        # then accumulate with start=False, making their order irrelevant.
        n_ycols = nb * NBLK
        for col0 in range(0, n_ycols, 512):
            w = min(512, n_ycols - col0)
            nc.tensor.matmul(
                out=y_ps[:, col0:col0 + w],
                lhsT=ol_sb[:],
                rhs=zrhs_sb[:, :w],
                start=True,
                stop=False,
            )

        d2_tiles = {}

        # warm up the PE p-state ramp (~3.4us of activity -> 2.4 GHz) with
        # junk matmuls while the geometry DMA is still in flight
        wp = ppool.tile([128, 128], f32, tag="warm", bufs=1)
        for _ in range(24):
            nc.tensor.matmul(
                out=wp[:],
                lhsT=ol_sb[:],
                rhs=zrhs_sb[:, 0:128],
                start=True,
                stop=True,
            )

        def emit_d2(c, ic):
            pt = ppool.tile([128, 512], f32, tag="d2psum")
            nc.tensor.matmul(
                out=pt[:],
                lhsT=lhs_sb[:, c * 128:(c + 1) * 128],
                rhs=rhs_sb[:, ic * 512:(ic + 1) * 512],
                start=True,
                stop=True,
            )
            # pt = -d2/2; clamp d2 >= 0  <=>  pt <= 0 (immediate scalar)
            nc.vector.tensor_scalar_min(
                d2_tiles[c][:, ic * 512:(ic + 1) * 512], pt[:], 0.0
            )

        d2_tiles[0] = d2pool.tile([128, m_out], f32, tag="d2s", name="d2s0")
        for ic in range(NIC):
            emit_d2(0, ic)

        # split the very first exp so ACT starts after only a quarter of
        # chunk 0's d2 matmuls instead of all 16
        split_first = (NIC % 4 == 0)

        for c in range(C):
            if c + 1 < C:
                d2_tiles[c + 1] = d2pool.tile(
                    [128, m_out], f32, tag="d2s", name=f"d2s{c + 1}"
                )
            if not (c == 0 and split_first):
                # absorber: advance ACT's observed DVE tick past this chunk's
                # clamps, so the real activations below carry at most 1 wait
                nc.scalar.copy(out=ascr[:], in_=d2_tiles[c][:, m_out - 1:m_out])
            for b in range(nb):
                col = c * nb + b
                e = epool.tile([128, m_out], f16, tag="e")
                if c == 0 and b == 0 and split_first:
                    q = m_out // 4
                    for qi in range(4):
                        nc.scalar.activation(
                            out=e[:, qi * q:(qi + 1) * q],
                            in_=d2_tiles[c][:, qi * q:(qi + 1) * q],
                            func=mybir.ActivationFunctionType.Exp,
                            bias=bi_sb[:, c:c + 1],
                            scale=b2_sb[:, col:col + 1],
                        )
                    nc.scalar.copy(
                        out=ascr[:], in_=d2_tiles[c][:, m_out - 1:m_out]
                    )
                else:
                    nc.scalar.activation(
                        out=e[:],
                        in_=d2_tiles[c][:],
                        func=mybir.ActivationFunctionType.Exp,
                        bias=bi_sb[:, c:c + 1],
                        scale=b2_sb[:, col:col + 1],
                    )
                # interleave next chunk's d2 early (before this basis' reduce
                # matmuls) so the PE computes it while the exp is in flight
                if c + 1 < C:
                    for ic in (2 * b, 2 * b + 1):
                        if ic < NIC:
                            emit_d2(c + 1, ic)
                for blk in range(NBLK):
                    nc.tensor.matmul(
                        out=y_ps[:, b * NBLK + blk: b * NBLK + blk + 1],
                        lhsT=e[:, blk * 128:(blk + 1) * 128],
                        rhs=ol_sb[:, 0:1],
                        start=False,
                        stop=False,
                    )
            del d2_tiles[c]

        # Close the accumulation groups: whole-bank +0 matmuls with stop=True.
        # The full-bank WAW overlap orders these after every real reduce matmul.
        for col0 in range(0, n_ycols, 512):
            w = min(512, n_ycols - col0)
            nc.tensor.matmul(
                out=y_ps[:, col0:col0 + w],
                lhsT=ol_sb[:],
                rhs=zrhs_sb[:, :w],
                start=False,
                stop=True,
            )

        y_sb = opool.tile([128, nb * NBLK], f32)
        nc.vector.tensor_copy(out=y_sb[:], in_=y_ps[:])
        nc.sync.dma_start(out=y_d.ap(), in_=y_sb[:])

    nc.compile()
    return nc


def _host_precompute(rho, gamma, coords, weights, out_coords, w1, b1, w2, b2):
    """Float64 host-side precompute of the tiny MLP and derived vectors."""
    rho = rho.astype(np.float64)
    gamma = gamma.astype(np.float64)
    coords64 = coords.astype(np.float64)
    weights64 = weights.astype(np.float64)
    oc64 = out_coords.astype(np.float64)
    w1, b1, w2, b2 = (a.astype(np.float64) for a in (w1, b1, w2, b2))

    def log_cosh(z):
        a = np.abs(z)
        return a + np.log1p(np.exp(-2.0 * a)) - LOG2

    def field_embed(x):
        return np.tanh(x @ w1 + b1) @ w2 + b2

    s2 = gamma / (4.0 * (3.0 * np.pi ** 2) ** (2.0 / 3.0) * rho ** (8.0 / 3.0))
    x = np.log(s2 + EPS)[:, None]
    exponent = log_cosh(field_embed(x))                      # (N, NB)
    heg = log_cosh(field_embed(np.zeros((1, 1)))) ** 1.5     # (1, NB)
    beta = np.pi * (rho[:, None] / 2.0) ** (2.0 / 3.0) * exponent  # (N, NB)
    wrho = weights64 * rho                                   # (N,)
    rj2 = (coords64 ** 2).sum(axis=1)                        # (N,)
    ri2 = (oc64 ** 2).sum(axis=1)                            # (M,)
    lnw = np.log(SCALE * np.maximum(wrho, 1e-300))           # (N,)
    return beta, wrho, heg[0], rj2, ri2, lnw, coords64, oc64


def kernel(rho, gamma, coords, weights, out_coords, w1, b1, w2, b2):
    from concourse.bass_utils import run_bass_kernel_spmd

    n_src = coords.shape[0]
    m_out = out_coords.shape[0]
    nb = w2.shape[1]
    n_src_pc = n_src // N_CORES
    C = n_src_pc // 128
    NBLK = m_out // 128

    beta, wrho, heg, rj2, ri2, lnw, coords64, oc64 = _host_precompute(
        rho, gamma, coords, weights, out_coords, w1, b1, w2, b2
    )

    key = (n_src_pc, m_out, nb)
    if key not in _CACHE:
        _CACHE[key] = _build_nc(n_src_pc, m_out, nb)
    nc = _CACHE[key]

    rhs_aug = np.concatenate(
        [oc64.T, np.ones((1, m_out)), (-0.5 * ri2)[None, :]], axis=0
    )                                                        # (5, M)

    in_maps = []
    for k in range(N_CORES):
        js = slice(k * n_src_pc, (k + 1) * n_src_pc)
        lhs_aug = np.concatenate(
            [coords64[js].T, (-0.5 * rj2[js])[None, :], np.ones((1, n_src_pc))],
            axis=0,
        )                                                    # (5, n_pc)
        geom = np.concatenate([rhs_aug, lhs_aug], axis=1).astype(np.float32)
        # (128, C*nb) with column c*nb+b -> source j = k*n_pc + c*128 + p
        beta2 = (2.0 * beta[js]).reshape(C, 128, nb).transpose(1, 0, 2).reshape(
            128, C * nb
        )
        lnwv = lnw[js].reshape(C, 128).T                     # (128, C)
        coef = np.concatenate([beta2, lnwv], axis=1).astype(np.float32)
        in_maps.append(
            {
                "geom": np.ascontiguousarray(geom),
                "coef": np.ascontiguousarray(coef),
            }
        )

    res = run_bass_kernel_spmd(nc, in_maps, core_ids=list(range(N_CORES)))
    _LAST_RUN["nc"] = nc
    _LAST_RUN["in_maps"] = in_maps
    _LAST_RUN["results"] = res

    ytot = np.zeros((m_out, nb), dtype=np.float64)
    for k in range(N_CORES):
        arr = res.results[k]["yout"]                         # (128, nb*NBLK)
        part = arr.reshape(128, nb, NBLK).transpose(2, 0, 1).reshape(m_out, nb)
        ytot += part.astype(np.float64)
    y = ytot * heg[None, :] / SCALE
    return y.astype(np.float32)


# revision 28
# speedup vs baseline: 1.0236x; 1.0211x over previous
"""Trainium2 Bass kernel for the CoarseGraining problem.

Computes y[i, b] = heg[b] * sum_j wrho[j] * exp(-beta[j, b] * d2[i, j])
with d2 the pairwise squared distances between out_coords (i) and coords (j).

Strategy (8 NeuronCores, SPMD):
  - Shard the j (source) axis: each core owns 1024 sources and reduces them
    over ALL 8192 output points; host sums the 8 partial results at the end.
  - Per-core layout: partitions = j (chunks of 128), free axis = i.
  - The tiny MLP (beta), wrho, heg and the squared norms are precomputed on
    the host in float64; beta/wrho enter the device kernel only through the
    per-partition scale/bias operands of the activation instruction.
  - Device pipeline per chunk c (128 sources):
      1. PE:  K=4 fp32 matmul  P'[j, i] = c_j . x_i - ri2[i]/2    (16 x N=512)
      2. DVE: clamp  d2s = min(P', rj2[j]/2)  (== enforcing d2 >= 0)
      3. ACT (per basis b): E = exp(2*beta[j,b] * d2s + bias[j,b]) -> fp16
         where bias = -beta*rj2 + ln(1024*wrho); ONE (128, 8192) op.
      4. PE:  reduce over j: lhsT = E[:, 128-block] (stationary), rhs = ones
         column -> psum column y[(b, blk)], accumulated over chunks in PSUM.
"""

import numpy as np
from contextlib import ExitStack

N_CORES = 8
N_SRC = 8192
M_OUT = 8192
NB = 16
EPS = 1e-4
LOG2 = 0.6931471805599453
SCALE = 1024.0

_CACHE = {}
_LAST_RUN = {}


def _build_nc(n_src_pc, m_out, nb):
    import concourse.bass as bass
    import concourse.tile as tile
    from concourse import bacc, mybir

    f32 = mybir.dt.float32
    f32r = mybir.dt.float32r
    f16 = mybir.dt.float16

    C = n_src_pc // 128          # j-chunks per core
    NIC = m_out // 512           # d2 matmul slices
    NBLK = m_out // 128          # reduce blocks (i blocks of 128)

    nc = bacc.Bacc("TRN2", target_bir_lowering=False, debug=False)
    # geom cols 0:m_out   -> rhs rows [x; y; z; 1; -ri2/2]
    # geom cols m_out:    -> lhsT rows [cx; cy; cz; -rj2/2; 1]
    geom_d = nc.dram_tensor("geom", [5, m_out + n_src_pc], f32, kind="ExternalInput")
    # coef: [2*beta (C*nb) | ln(SCALE*wrho) (C)] per 128-partition layout
    coef_d = nc.dram_tensor("coef", [128, C * nb + C], f32, kind="ExternalInput")
    y_d = nc.dram_tensor("yout", [128, nb * NBLK], f32, kind="ExternalOutput")

    with ExitStack() as ctx:
        tc = ctx.enter_context(tile.TileContext(nc))
        consts = ctx.enter_context(tc.tile_pool(name="consts", bufs=1))
        d2pool = ctx.enter_context(tc.tile_pool(name="d2p", bufs=2))
        epool = ctx.enter_context(tc.tile_pool(name="ep", bufs=4))
        ppool = ctx.enter_context(tc.tile_pool(name="pp", bufs=5, space="PSUM"))
        ypool = ctx.enter_context(tc.tile_pool(name="yp", bufs=1, space="PSUM"))
        opool = ctx.enter_context(tc.tile_pool(name="op", bufs=1))

        geom_sb = consts.tile([5, m_out + n_src_pc], f32)
        nc.sync.dma_start(out=geom_sb[:], in_=geom_d.ap())
        rhs_sb = geom_sb[:, 0:m_out]
        lhs_sb = geom_sb[:, m_out:m_out + n_src_pc]
        coef_sb = consts.tile([128, C * nb + C], f32)
        nc.sync.dma_start(out=coef_sb[:], in_=coef_d.ap())
        b2_sb = coef_sb[:, 0:C * nb]
        bi_sb = coef_sb[:, C * nb:C * nb + C]
        # ones tile memset happens below; emit an ACT op that depends on
        # nothing slow so the exp table load fires at t~0, then a scratch
        # ACT op per dependency source to absorb sync waits (AP-operand
        # instructions only have a single sync-wait slot in the encoding)
        ascr = consts.tile([128, 1], f32)

        # ones tile: column 0 is the reduce-matmul rhs; the whole tile is the
        # dummy zeroing matmul's lhsT (so later matmuls never re-wait on it).
        ol_sb = consts.tile([128, 128], f16)
        nc.vector.memset(ol_sb[:], 1.0)
        zrhs_sb = consts.tile([128, min(512, nb * NBLK)], f16)
        nc.vector.memset(zrhs_sb[:], 0.0)
        nc.scalar.copy(out=ascr[:], in_=ol_sb[:, 0:1])   # early table load
        nc.scalar.copy(out=ascr[:], in_=coef_sb[:, 0:1])  # absorb coef DMA wait

        y_ps = ypool.tile([128, nb * NBLK], f32)

        # Zero-initialize y_ps with whole-bank dummy matmuls (start=True
        # clears has_written for the entire bank); all real reduce matmuls
        # then accumulate with start=False, making their order irrelevant.
        n_ycols = nb * NBLK
        for col0 in range(0, n_ycols, 512):
            w = min(512, n_ycols - col0)
            nc.tensor.matmul(
                out=y_ps[:, col0:col0 + w],
                lhsT=ol_sb[:],
                rhs=zrhs_sb[:, :w],
                start=True,
                stop=False,
            )

        d2_tiles = {}

        # warm up the PE p-state ramp (~3.4us of activity -> 2.4 GHz) with
        # junk matmuls while the geometry DMA is still in flight
        wp = ppool.tile([128, min(512, nb * NBLK)], f32, tag="warm", bufs=1)
        for _ in range(20):
            nc.tensor.matmul(
                out=wp[:],
                lhsT=ol_sb[:],
                rhs=zrhs_sb[:],
                start=True,
                stop=True,
            )

        def emit_d2(c, ic):
            pt = ppool.tile([128, 512], f32, tag="d2psum")
            nc.tensor.matmul(
                out=pt[:],
                lhsT=lhs_sb[:, c * 128:(c + 1) * 128],
                rhs=rhs_sb[:, ic * 512:(ic + 1) * 512],
                start=True,
                stop=True,
            )
            # pt = -d2/2; clamp d2 >= 0  <=>  pt <= 0 (immediate scalar)
            nc.vector.tensor_scalar_min(
                d2_tiles[c][:, ic * 512:(ic + 1) * 512], pt[:], 0.0
            )

        d2_tiles[0] = d2pool.tile([128, m_out], f32, tag="d2s", name="d2s0")
        for ic in range(NIC):
            emit_d2(0, ic)

        # split the very first exp so ACT starts after only a quarter of
        # chunk 0's d2 matmuls instead of all 16
        split_first = (NIC % 4 == 0)

        for c in range(C):
            if c + 1 < C:
                d2_tiles[c + 1] = d2pool.tile(
                    [128, m_out], f32, tag="d2s", name=f"d2s{c + 1}"
                )
            if not (c == 0 and split_first):
                # absorber: advance ACT's observed DVE tick past this chunk's
                # clamps, so the real activations below carry at most 1 wait
                nc.scalar.copy(out=ascr[:], in_=d2_tiles[c][:, m_out - 1:m_out])
            for b in range(nb):
                col = c * nb + b
                e = epool.tile([128, m_out], f16, tag="e")
                if c == 0 and b == 0 and split_first:
                    q = m_out // 4
                    for qi in range(4):
                        nc.scalar.activation(
                            out=e[:, qi * q:(qi + 1) * q],
                            in_=d2_tiles[c][:, qi * q:(qi + 1) * q],
                            func=mybir.ActivationFunctionType.Exp,
                            bias=bi_sb[:, c:c + 1],
                            scale=b2_sb[:, col:col + 1],
                        )
                    nc.scalar.copy(
                        out=ascr[:], in_=d2_tiles[c][:, m_out - 1:m_out]
                    )
                else:
                    nc.scalar.activation(
                        out=e[:],
                        in_=d2_tiles[c][:],
                        func=mybir.ActivationFunctionType.Exp,
                        bias=bi_sb[:, c:c + 1],
                        scale=b2_sb[:, col:col + 1],
                    )
                # interleave next chunk's d2 early (before this basis' reduce
                # matmuls) so the PE computes it while the exp is in flight
                if c + 1 < C:
                    for ic in (2 * b, 2 * b + 1):
                        if ic < NIC:
                            emit_d2(c + 1, ic)
                for blk in range(NBLK):
                    nc.tensor.matmul(
                        out=y_ps[:, b * NBLK + blk: b * NBLK + blk + 1],
                        lhsT=e[:, blk * 128:(blk + 1) * 128],
                        rhs=ol_sb[:, 0:1],
                        start=False,
                        stop=False,
                    )
            del d2_tiles[c]

        # Close the accumulation groups: whole-bank +0 matmuls with stop=True.
        # The full-bank WAW overlap orders these after every real reduce matmul.
        for col0 in range(0, n_ycols, 512):
            w = min(512, n_ycols - col0)
            nc.tensor.matmul(
                out=y_ps[:, col0:col0 + w],
                lhsT=ol_sb[:],
                rhs=zrhs_sb[:, :w],
                start=False,
                stop=True,
            )

        y_sb = opool.tile([128, nb * NBLK], f32)
        nc.vector.tensor_copy(out=y_sb[:], in_=y_ps[:])
        nc.sync.dma_start(out=y_d.ap(), in_=y_sb[:])

    nc.compile()
    return nc


def _host_precompute(rho, gamma, coords, weights, out_coords, w1, b1, w2, b2):
    """Float64 host-side precompute of the tiny MLP and derived vectors."""
    rho = rho.astype(np.float64)
    gamma = gamma.astype(np.float64)
    coords64 = coords.astype(np.float64)
    weights64 = weights.astype(np.float64)
    oc64 = out_coords.astype(np.float64)
    w1, b1, w2, b2 = (a.astype(np.float64) for a in (w1, b1, w2, b2))

    def log_cosh(z):
        a = np.abs(z)
        return a + np.log1p(np.exp(-2.0 * a)) - LOG2

    def field_embed(x):
        return np.tanh(x @ w1 + b1) @ w2 + b2

    s2 = gamma / (4.0 * (3.0 * np.pi ** 2) ** (2.0 / 3.0) * rho ** (8.0 / 3.0))
    x = np.log(s2 + EPS)[:, None]
    exponent = log_cosh(field_embed(x))                      # (N, NB)
    heg = log_cosh(field_embed(np.zeros((1, 1)))) ** 1.5     # (1, NB)
    beta = np.pi * (rho[:, None] / 2.0) ** (2.0 / 3.0) * exponent  # (N, NB)
    wrho = weights64 * rho                                   # (N,)
    rj2 = (coords64 ** 2).sum(axis=1)                        # (N,)
    ri2 = (oc64 ** 2).sum(axis=1)                            # (M,)
    lnw = np.log(SCALE * np.maximum(wrho, 1e-300))           # (N,)
    return beta, wrho, heg[0], rj2, ri2, lnw, coords64, oc64


def kernel(rho, gamma, coords, weights, out_coords, w1, b1, w2, b2):
    from concourse.bass_utils import run_bass_kernel_spmd

    n_src = coords.shape[0]
    m_out = out_coords.shape[0]
    nb = w2.shape[1]
    n_src_pc = n_src // N_CORES
    C = n_src_pc // 128
    NBLK = m_out // 128

    beta, wrho, heg, rj2, ri2, lnw, coords64, oc64 = _host_precompute(
        rho, gamma, coords, weights, out_coords, w1, b1, w2, b2
    )

    key = (n_src_pc, m_out, nb)
    if key not in _CACHE:
        _CACHE[key] = _build_nc(n_src_pc, m_out, nb)
    nc = _CACHE[key]

    rhs_aug = np.concatenate(
        [oc64.T, np.ones((1, m_out)), (-0.5 * ri2)[None, :]], axis=0
    )                                                        # (5, M)

    in_maps = []
    for k in range(N_CORES):
        js = slice(k * n_src_pc, (k + 1) * n_src_pc)
        lhs_aug = np.concatenate(
            [coords64[js].T, (-0.5 * rj2[js])[None, :], np.ones((1, n_src_pc))],
            axis=0,
        )                                                    # (5, n_pc)
        geom = np.concatenate([rhs_aug, lhs_aug], axis=1).astype(np.float32)
        # (128, C*nb) with column c*nb+b -> source j = k*n_pc + c*128 + p
        beta2 = (2.0 * beta[js]).reshape(C, 128, nb).transpose(1, 0, 2).reshape(
            128, C * nb
        )
        lnwv = lnw[js].reshape(C, 128).T                     # (128, C)
        coef = np.concatenate([beta2, lnwv], axis=1).astype(np.float32)
        in_maps.append(
            {
                "geom": np.ascontiguousarray(geom),
                "coef": np.ascontiguousarray(coef),
            }
        )

    res = run_bass_kernel_spmd(nc, in_maps, core_ids=list(range(N_CORES)))
    _LAST_RUN["nc"] = nc
    _LAST_RUN["in_maps"] = in_maps
    _LAST_RUN["results"] = res

    ytot = np.zeros((m_out, nb), dtype=np.float64)
    for k in range(N_CORES):
        arr = res.results[k]["yout"]                         # (128, nb*NBLK)
        part = arr.reshape(128, nb, NBLK).transpose(2, 0, 1).reshape(m_out, nb)
        ytot += part.astype(np.float64)
    y = ytot * heg[None, :] / SCALE
    return y.astype(np.float32)


# revision 33
# speedup vs baseline: 1.0414x; 1.0174x over previous
"""Trainium2 Bass kernel for the CoarseGraining problem.

Computes y[i, b] = heg[b] * sum_j wrho[j] * exp(-beta[j, b] * d2[i, j])
with d2 the pairwise squared distances between out_coords (i) and coords (j).

Strategy (8 NeuronCores, SPMD):
  - Shard the j (source) axis: each core owns 1024 sources and reduces them
    over ALL 8192 output points; host sums the 8 partial results at the end.
  - Per-core layout: partitions = j (chunks of 128), free axis = i.
  - The tiny MLP (beta), wrho, heg and the squared norms are precomputed on
    the host in float64; beta/wrho enter the device kernel only through the
    per-partition scale/bias operands of the activation instruction.
  - Device pipeline per chunk c (128 sources):
      1. PE:  K=4 fp32 matmul  P'[j, i] = c_j . x_i - ri2[i]/2    (16 x N=512)
      2. DVE: clamp  d2s = min(P', rj2[j]/2)  (== enforcing d2 >= 0)
      3. ACT (per basis b): E = exp(2*beta[j,b] * d2s + bias[j,b]) -> fp16
         where bias = -beta*rj2 + ln(1024*wrho); ONE (128, 8192) op.
      4. PE:  reduce over j: lhsT = E[:, 128-block] (stationary), rhs = ones
         column -> psum column y[(b, blk)], accumulated over chunks in PSUM.
"""

import numpy as np
from contextlib import ExitStack

N_CORES = 8
N_SRC = 8192
M_OUT = 8192
NB = 16
EPS = 1e-4
LOG2 = 0.6931471805599453
SCALE = 1024.0

_CACHE = {}
_LAST_RUN = {}


def _build_nc(n_src_pc, m_out, nb):
    import concourse.bass as bass
    import concourse.tile as tile
    from concourse import bacc, mybir

    f32 = mybir.dt.float32
    f16 = mybir.dt.float16
    bf16 = mybir.dt.bfloat16

    C = n_src_pc // 128          # j-chunks per core
    NIC = m_out // 512           # d2 matmul slices
    NBLK = m_out // 128          # reduce blocks (i blocks of 128)

    nc = bacc.Bacc("TRN2", target_bir_lowering=False, debug=False)
    # geom: 24 bf16 rows encoding the exact fp32 dot products via 3-way
    # bf16 splits (bf16 runs the PE at 4x the fp32 matmul rate):
    #   rows 6k..6k+5 (dim k): lhsT [c1,c1,c1,c2,c2,c3] x rhs [x1,x2,x3,x1,x2,x1]
    #   rows 18-20: lhsT [r1,r2,r3] x rhs [1,1,1]     (r = -rj2/2 split)
    #   rows 21-23: lhsT [1,1,1] x rhs [s1,s2,s3]     (s = -ri2/2 split)
    geom_d = nc.dram_tensor("geom", [24, m_out + n_src_pc], bf16, kind="ExternalInput")
    # coef: [2*beta (C*nb) | ln(SCALE*wrho) (C)] per 128-partition layout
    coef_d = nc.dram_tensor("coef", [128, C * nb + C], f32, kind="ExternalInput")
    y_d = nc.dram_tensor("yout", [128, nb * NBLK], f32, kind="ExternalOutput")

    with ExitStack() as ctx:
        tc = ctx.enter_context(tile.TileContext(nc))
        consts = ctx.enter_context(tc.tile_pool(name="consts", bufs=1))
        d2pool = ctx.enter_context(tc.tile_pool(name="d2p", bufs=2))
        epool = ctx.enter_context(tc.tile_pool(name="ep", bufs=4))
        ppool = ctx.enter_context(tc.tile_pool(name="pp", bufs=5, space="PSUM"))
        ypool = ctx.enter_context(tc.tile_pool(name="yp", bufs=1, space="PSUM"))
        opool = ctx.enter_context(tc.tile_pool(name="op", bufs=1))

        geom_sb = consts.tile([24, m_out + n_src_pc], bf16)
        nc.sync.dma_start(out=geom_sb[:], in_=geom_d.ap())
        rhs_sb = geom_sb[:, 0:m_out]
        lhs_sb = geom_sb[:, m_out:m_out + n_src_pc]
        coef_sb = consts.tile([128, C * nb + C], f32)
        nc.sync.dma_start(out=coef_sb[:], in_=coef_d.ap())
        b2_sb = coef_sb[:, 0:C * nb]
        bi_sb = coef_sb[:, C * nb:C * nb + C]
        # ones tile memset happens below; emit an ACT op that depends on
        # nothing slow so the exp table load fires at t~0, then a scratch
        # ACT op per dependency source to absorb sync waits (AP-operand
        # instructions only have a single sync-wait slot in the encoding)
        ascr = consts.tile([128, 1], f32)

        # ones tile: column 0 is the reduce-matmul rhs; the whole tile is the
        # dummy zeroing matmul's lhsT (so later matmuls never re-wait on it).
        ol_sb = consts.tile([128, 128], f16)
        nc.vector.memset(ol_sb[:], 1.0)
        zrhs_sb = consts.tile([128, min(512, nb * NBLK)], f16)
        nc.vector.memset(zrhs_sb[:], 0.0)
        nc.scalar.copy(out=ascr[:], in_=ol_sb[:, 0:1])   # early table load
        nc.scalar.copy(out=ascr[:], in_=coef_sb[:, 0:1])  # absorb coef DMA wait

        y_ps = ypool.tile([128, nb * NBLK], f32)

        # Zero-initialize y_ps with whole-bank dummy matmuls (start=True
        # clears has_written for the entire bank); all real reduce matmuls
        # then accumulate with start=False, making their order irrelevant.
        n_ycols = nb * NBLK
        for col0 in range(0, n_ycols, 512):
            w = min(512, n_ycols - col0)
            nc.tensor.matmul(
                out=y_ps[:, col0:col0 + w],
                lhsT=ol_sb[:],
                rhs=zrhs_sb[:, :w],
                start=True,
                stop=False,
            )

        d2_tiles = {}

        # warm up the PE p-state ramp (~3.4us of activity -> 2.4 GHz) with
        # junk matmuls while the geometry DMA is still in flight
        wp = ppool.tile([128, min(512, nb * NBLK)], f32, tag="warm", bufs=1)
        for _ in range(20):
            nc.tensor.matmul(
                out=wp[:],
                lhsT=ol_sb[:],
                rhs=zrhs_sb[:],
                start=True,
                stop=True,
            )

        def emit_d2(c, ic):
            pt = ppool.tile([128, 512], f32, tag="d2psum")
            nc.tensor.matmul(
                out=pt[:],
                lhsT=lhs_sb[:, c * 128:(c + 1) * 128],
                rhs=rhs_sb[:, ic * 512:(ic + 1) * 512],
                start=True,
                stop=True,
            )
            # pt = -d2/2; clamp d2 >= 0  <=>  pt <= 0 (immediate scalar)
            nc.vector.tensor_scalar_min(
                d2_tiles[c][:, ic * 512:(ic + 1) * 512], pt[:], 0.0
            )

        d2_tiles[0] = d2pool.tile([128, m_out], f32, tag="d2s", name="d2s0")
        for ic in range(NIC):
            emit_d2(0, ic)

        # split the very first exp so ACT starts after only a quarter of
        # chunk 0's d2 matmuls instead of all 16
        split_first = (NIC % 4 == 0)

        for c in range(C):
            if c + 1 < C:
                d2_tiles[c + 1] = d2pool.tile(
                    [128, m_out], f32, tag="d2s", name=f"d2s{c + 1}"
                )
            if not (c == 0 and split_first):
                # absorber: advance ACT's observed DVE tick past this chunk's
                # clamps, so the real activations below carry at most 1 wait
                nc.scalar.copy(out=ascr[:], in_=d2_tiles[c][:, m_out - 1:m_out])
            for b in range(nb):
                col = c * nb + b
                e = epool.tile([128, m_out], f16, tag="e")
                if c == 0 and b == 0 and split_first:
                    q = m_out // 4
                    for qi in range(4):
                        nc.scalar.activation(
                            out=e[:, qi * q:(qi + 1) * q],
                            in_=d2_tiles[c][:, qi * q:(qi + 1) * q],
                            func=mybir.ActivationFunctionType.Exp,
                            bias=bi_sb[:, c:c + 1],
                            scale=b2_sb[:, col:col + 1],
                        )
                    nc.scalar.copy(
                        out=ascr[:], in_=d2_tiles[c][:, m_out - 1:m_out]
                    )
                else:
                    nc.scalar.activation(
                        out=e[:],
                        in_=d2_tiles[c][:],
                        func=mybir.ActivationFunctionType.Exp,
                        bias=bi_sb[:, c:c + 1],
                        scale=b2_sb[:, col:col + 1],
                    )
                # interleave next chunk's d2 early (before this basis' reduce
                # matmuls) so the PE computes it while the exp is in flight
                if c + 1 < C:
                    for ic in (2 * b, 2 * b + 1):
                        if ic < NIC:
                            emit_d2(c + 1, ic)
                for blk in range(NBLK):
                    nc.tensor.matmul(
                        out=y_ps[:, b * NBLK + blk: b * NBLK + blk + 1],
                        lhsT=e[:, blk * 128:(blk + 1) * 128],
                        rhs=ol_sb[:, 0:1],
                        start=False,
                        stop=False,
                    )
            del d2_tiles[c]

        # Close the accumulation groups: whole-bank +0 matmuls with stop=True.
        # The full-bank WAW overlap orders these after every real reduce matmul.
        for col0 in range(0, n_ycols, 512):
            w = min(512, n_ycols - col0)
            nc.tensor.matmul(
                out=y_ps[:, col0:col0 + w],
                lhsT=ol_sb[:],
                rhs=zrhs_sb[:, :w],
                start=False,
                stop=True,
            )

        y_sb = opool.tile([128, nb * NBLK], f32)
        nc.vector.tensor_copy(out=y_sb[:], in_=y_ps[:])
        nc.sync.dma_start(out=y_d.ap(), in_=y_sb[:])

    nc.compile()
    return nc


def _bsplit3(v):
    """Split f32 values into three bf16 parts summing exactly to the f32."""
    import ml_dtypes

    bf = ml_dtypes.bfloat16
    v32 = np.asarray(v, dtype=np.float32)
    p1 = v32.astype(bf)
    r = v32 - p1.astype(np.float32)
    p2 = r.astype(bf)
    r2 = r - p2.astype(np.float32)
    p3 = r2.astype(bf)
    return p1, p2, p3


def _pack_geom(coords_side, dot_side, nsq_half_neg):
    """Build 24 bf16 rows for one side of the split d2 matmul.

    coords_side: (n, 3) float64 point coordinates for this side
    dot_side: 'lhs' or 'rhs' (which split pattern to emit)
    nsq_half_neg: (n,) float64, the -|p|^2/2 values for this side
    """
    import ml_dtypes

    bf = ml_dtypes.bfloat16
    n = coords_side.shape[0]
    rows = np.zeros((24, n), dtype=bf)
    for k in range(3):
        p1, p2, p3 = _bsplit3(coords_side[:, k])
        if dot_side == "lhs":
            rows[6 * k + 0] = p1
            rows[6 * k + 1] = p1
            rows[6 * k + 2] = p1
            rows[6 * k + 3] = p2
            rows[6 * k + 4] = p2
            rows[6 * k + 5] = p3
        else:
            rows[6 * k + 0] = p1
            rows[6 * k + 1] = p2
            rows[6 * k + 2] = p3
            rows[6 * k + 3] = p1
            rows[6 * k + 4] = p2
            rows[6 * k + 5] = p1
    q1, q2, q3 = _bsplit3(nsq_half_neg)
    one = np.ones(n, dtype=bf)
    if dot_side == "lhs":
        rows[18], rows[19], rows[20] = q1, q2, q3
        rows[21] = rows[22] = rows[23] = one
    else:
        rows[18] = rows[19] = rows[20] = one
        rows[21], rows[22], rows[23] = q1, q2, q3
    return rows


def _host_precompute(rho, gamma, coords, weights, out_coords, w1, b1, w2, b2):
    """Float64 host-side precompute of the tiny MLP and derived vectors."""
    rho = rho.astype(np.float64)
    gamma = gamma.astype(np.float64)
    coords64 = coords.astype(np.float64)
    weights64 = weights.astype(np.float64)
    oc64 = out_coords.astype(np.float64)
    w1, b1, w2, b2 = (a.astype(np.float64) for a in (w1, b1, w2, b2))

    def log_cosh(z):
        a = np.abs(z)
        return a + np.log1p(np.exp(-2.0 * a)) - LOG2

    def field_embed(x):
        return np.tanh(x @ w1 + b1) @ w2 + b2

    s2 = gamma / (4.0 * (3.0 * np.pi ** 2) ** (2.0 / 3.0) * rho ** (8.0 / 3.0))
    x = np.log(s2 + EPS)[:, None]
    exponent = log_cosh(field_embed(x))                      # (N, NB)
    heg = log_cosh(field_embed(np.zeros((1, 1)))) ** 1.5     # (1, NB)
    beta = np.pi * (rho[:, None] / 2.0) ** (2.0 / 3.0) * exponent  # (N, NB)
    wrho = weights64 * rho                                   # (N,)
    rj2 = (coords64 ** 2).sum(axis=1)                        # (N,)
    ri2 = (oc64 ** 2).sum(axis=1)                            # (M,)
    lnw = np.log(SCALE * np.maximum(wrho, 1e-300))           # (N,)
    return beta, wrho, heg[0], rj2, ri2, lnw, coords64, oc64


def kernel(rho, gamma, coords, weights, out_coords, w1, b1, w2, b2):
    from concourse.bass_utils import run_bass_kernel_spmd

    n_src = coords.shape[0]
    m_out = out_coords.shape[0]
    nb = w2.shape[1]
    n_src_pc = n_src // N_CORES
    C = n_src_pc // 128
    NBLK = m_out // 128

    beta, wrho, heg, rj2, ri2, lnw, coords64, oc64 = _host_precompute(
        rho, gamma, coords, weights, out_coords, w1, b1, w2, b2
    )

    key = (n_src_pc, m_out, nb)
    if key not in _CACHE:
        _CACHE[key] = _build_nc(n_src_pc, m_out, nb)
    nc = _CACHE[key]

    rhs_aug = _pack_geom(oc64, "rhs", -0.5 * ri2)            # (24, M) bf16

    in_maps = []
    for k in range(N_CORES):
        js = slice(k * n_src_pc, (k + 1) * n_src_pc)
        lhs_aug = _pack_geom(coords64[js], "lhs", -0.5 * rj2[js])  # (24, n_pc)
        geom = np.concatenate([rhs_aug, lhs_aug], axis=1)
        # (128, C*nb) with column c*nb+b -> source j = k*n_pc + c*128 + p
        beta2 = (2.0 * beta[js]).reshape(C, 128, nb).transpose(1, 0, 2).reshape(
            128, C * nb
        )
        lnwv = lnw[js].reshape(C, 128).T                     # (128, C)
        coef = np.concatenate([beta2, lnwv], axis=1).astype(np.float32)
        in_maps.append(
            {
                "geom": np.ascontiguousarray(geom),
                "coef": np.ascontiguousarray(coef),
            }
        )

    res = run_bass_kernel_spmd(nc, in_maps, core_ids=list(range(N_CORES)))
    _LAST_RUN["nc"] = nc
    _LAST_RUN["in_maps"] = in_maps
    _LAST_RUN["results"] = res

    ytot = np.zeros((m_out, nb), dtype=np.float64)
    for k in range(N_CORES):
        arr = res.results[k]["yout"]                         # (128, nb*NBLK)
        part = arr.reshape(128, nb, NBLK).transpose(2, 0, 1).reshape(m_out, nb)
        ytot += part.astype(np.float64)
    y = ytot * heg[None, :] / SCALE
    return y.astype(np.float32)
